# revision 4
# baseline (speedup 1.0000x reference)
"""KAN layer (Catmull-Rom spline edges) as a single-matmul Trainium2 kernel.

Math:
  out[n,o] = sum_j w[o,j] * s_oj(x[n,j]) + bias[o],  s_oj = Catmull-Rom spline
  with K=8 uniform knots on [-1,1].  Each edge spline is decomposed into
  15 atom-chunks (near-side truncated-power basis; 5-tap stencils annihilate
  cubics so the decomposition is well-conditioned):
      out = sum_c  Acol_c^T @ H_c   + bias (added in the PSUM->SBUF copy)
  H atoms: xc, xc^2, xc^3, z_s^2, z_s^3 (s=1..6)
      z_s = min(xc - m'_s, 0) for s<=3, max(xc - m'_s, 0) for s>=4,
            m'_s = (s-3.5)/3.5
  Boundary atoms hD=(xc+1)z1^2, hE=z6^2(3.5 z6-1) are folded into the z1/z6
  square+cube chunks (exact identities on the truncated supports).
  A-side (pure weight prepack) on host in fp16; x is clamped to [-1,1] and
  cast fp16 on host, so the device computes only the z/power atoms.
  Data-parallel over N across 8 NeuronCores.

Perf structure (v2):
  - x [128,128]f16 DMA'd first on the ACT HWDGE queue, then A-part1
    (9 chunks + bias col) on the same queue; A-part2 (6 chunks) rides the
    SP HWDGE queue in parallel -> both queues stream concurrently.
  - No Activation-engine ops at all -> no 1.3us ACT table load.
  - Atom work split DVE (z1..z5, squares, cubes, final copy) / GPSIMD
    (xc2, z6, xc3, z6^2, z6^3).
  - bias column rides the A transfer and is added by the DVE during the
    PSUM->SBUF copy (no rank-1 bias matmul on the critical path).
  - fp16 output DMA (host upcasts to f32); quantization ~5e-4 rel.
"""
import numpy as np
from math import comb

N, D_IN, D_OUT, K = 1024, 128, 128, 8
N_CORES = 8
N_LOC = N // N_CORES
N_CHUNKS = 15

_A_COEF = {-2: 0.5, -1: -2.0, 0: 3.0, 1: -2.0, 2: 0.5}
_B_COEF = {-2: -0.5, -1: 1.0, 0: 0.0, 1: -1.0, 2: 0.5}

# build order: [p1,p2,p3, s1..s6, c1..c6]
# emission order (matmul stream / acat column order):
#   [p1,p2,p3, s1,s2,s3,s4,s5, c1,c2,c3,c4,c5, s6, c6]
_EMIT = [0, 1, 2, 3, 4, 5, 6, 7, 9, 10, 11, 12, 13, 8, 14]
_NC_A1 = 9                      # chunks 0..8 of emission order ride part 1
_BIAS_COL = _NC_A1 * D_OUT      # bias column index inside acat
_PAD = 8                        # bias col + 7 pad cols, keeps 16B alignment
_A1_W = _NC_A1 * D_OUT + _PAD   # part-1 width (1160)
_ACAT_W = N_CHUNKS * D_OUT + _PAD  # 1928

_STATE = {}


def _chunk_cols(c):
    """acat column range of emission-chunk c."""
    if c < _NC_A1:
        s = c * D_OUT
    else:
        s = _A1_W + (c - _NC_A1) * D_OUT
    return s, s + D_OUT


def _poly_xc(s, p):
    """coeffs of (t-s)^p in powers of xc (const..xc^3), t = 3.5*xc + 3.5."""
    c = np.zeros(4)
    for i in range(p + 1):
        c[i] = comb(p, i) * (3.5 ** i) * ((3.5 - s) ** (p - i))
    return c


def _prepack(coeffs, weights, bias):
    """Host weight prepack -> acat fp16 [j, 1928] (emission order + bias)."""
    Ap = (coeffs.astype(np.float64) * weights.astype(np.float64)[:, :, None]
          ).transpose(1, 2, 0)                                   # [j,k,o]
    poly = np.zeros((4, D_IN, D_OUT))
    cube = np.zeros((6, D_IN, D_OUT))
    sq = np.zeros((6, D_IN, D_OUT))
    for k in range(K):
        for r in (-2, -1, 0, 1, 2):
            s = k + r
            ar, br = _A_COEF[r], _B_COEF[r]
            if s >= 7:
                continue
            if s <= 3:
                # a(t-s)_+^3 + b(t-s)_+^2
                #   = [a(t-s)^3 + b(t-s)^2] + a*(s-t)_+^3 - b*(s-t)_+^2
                poly += (ar * _poly_xc(s, 3) + br * _poly_xc(s, 2)
                         )[:, None, None] * Ap[:, k, :][None]
                if s >= 1:
                    # z_s = min(.,0): (s-t)_+^3 = -42.875 z^3,
                    #                 (s-t)_+^2 =  12.25 z^2
                    cube[s - 1] += -42.875 * ar * Ap[:, k, :]
                    sq[s - 1] += -12.25 * br * Ap[:, k, :]
            else:
                # z_s = max(.,0): (t-s)_+^3 = 42.875 z^3, (t-s)_+^2 = 12.25 z^2
                cube[s - 1] += 42.875 * ar * Ap[:, k, :]
                sq[s - 1] += 12.25 * br * Ap[:, k, :]
    D_col = -21.4375 * Ap[:, 0, :]       # atom (xc+1)*z_1^2
    E_col = 6.125 * Ap[:, 7, :]          # atom z_6^2*(3.5 z_6 - 1)
    # Fold the boundary atoms into existing chunks (exact identities on the
    # truncated supports):  hD = (xc+1) z1^2 = z1^3 + (1+m'_1) z1^2
    #                       hE = z6^2 (3.5 z6 - 1) = 3.5 z6^3 - z6^2
    m1 = (1 - 3.5) / 3.5
    cube[0] += D_col
    sq[0] += (1.0 + m1) * D_col
    cube[5] += 3.5 * E_col
    sq[5] -= E_col
    A = np.stack([poly[1], poly[2], poly[3], sq[0], sq[1], sq[2],
                  sq[3], sq[4], sq[5],
                  cube[0], cube[1], cube[2], cube[3], cube[4], cube[5]]
                 )                                               # [15,j,o]
    acat = np.zeros((D_IN, _ACAT_W), dtype=np.float16)
    for e, b in enumerate(_EMIT):
        c0, c1 = _chunk_cols(e)
        acat[:, c0:c1] = A[b].astype(np.float16)
    bias_full = (bias.astype(np.float64) + poly[0].sum(axis=0)
                 ).astype(np.float16)                            # [o]
    acat[:, _BIAS_COL] = bias_full                  # partition p holds bias[p]
    return np.ascontiguousarray(acat)


def _build_module():
    import concourse.bacc as bacc
    import concourse.bass as bass
    import concourse.mybir as mybir
    from concourse import tile

    f32 = mybir.dt.float32
    f16 = mybir.dt.float16
    Alu = mybir.AluOpType
    ts = bass.ts

    # Skip the all-engine barrier Bass.__init__ emits after the const-AP
    # memsets (~0.5us before the first DMA issue can happen).  This kernel
    # has no const-AP readers at all (immediates are instruction-encoded),
    # so the barrier protects nothing.
    _orig_barrier = bass.Bass.all_engine_barrier

    def _skip_once(self, *a, **k):
        bass.Bass.all_engine_barrier = _orig_barrier
        return None

    bass.Bass.all_engine_barrier = _skip_once
    try:
        nc = bacc.Bacc("TRN2", target_bir_lowering=False, debug=False,
                       enable_asserts=False, num_devices=N_CORES)
    finally:
        bass.Bass.all_engine_barrier = _orig_barrier
    xt = nc.dram_tensor("xt", [D_IN, N_LOC], f16, kind="ExternalInput").ap()
    acat = nc.dram_tensor("acat", [D_IN, _ACAT_W], f16,
                          kind="ExternalInput").ap()
    out_t = nc.dram_tensor("out_t", [D_OUT, N_LOC], f16,
                           kind="ExternalOutput").ap()

    mprime = [(s - 3.5) / 3.5 for s in range(1, 7)]
    HB = 5 * N_LOC  # z1..z5 block width

    with tile.TileContext(nc) as tc:
        with (
            tc.tile_pool(name="sbuf", bufs=1) as pool,
            tc.tile_pool(name="psum", bufs=1, space="PSUM") as ppool,
        ):
            x_sb = pool.tile([D_IN, N_LOC], f16, tag="x")
            a_sb = pool.tile([D_IN, _ACAT_W], f16, tag="acat")

            # ---- DMAs first.  x rides the ACT HWDGE queue and is issued
            # first (it gates all atom compute); A part 1 (chunks 0..8 of
            # the matmul stream + bias col) follows on the same queue.
            # A part 2 (chunks 9..14) rides the SP HWDGE queue in parallel.
            nc.scalar.dma_start(x_sb[:], xt[:])
            nc.scalar.dma_start(a_sb[:, 0:_A1_W], acat[:, 0:_A1_W])
            nc.sync.dma_start(a_sb[:, _A1_W:_ACAT_W], acat[:, _A1_W:_ACAT_W])

            # ---- x-side atoms (all fp16; x arrives pre-clamped) ----
            # DVE: z1..z5, sq(z1..z5), cu(z1..z5), final PSUM copy
            # GPSIMD: xc2, z6, xc3, sq6, cu6
            z = pool.tile([D_IN, 6 * N_LOC], f16, tag="z")
            z2 = pool.tile([D_IN, 6 * N_LOC], f16, tag="z2")
            z3 = pool.tile([D_IN, 6 * N_LOC], f16, tag="z3")
            xc2 = pool.tile([D_IN, N_LOC], f16, tag="xc2")
            xc3 = pool.tile([D_IN, N_LOC], f16, tag="xc3")

            for i in range(3):
                nc.vector.tensor_scalar(z[:, ts(i, N_LOC)], x_sb[:],
                                        mprime[i], 0.0, Alu.subtract, Alu.min)
            for i in (3, 4):
                nc.vector.tensor_scalar(z[:, ts(i, N_LOC)], x_sb[:],
                                        mprime[i], 0.0, Alu.subtract, Alu.max)
            nc.vector.tensor_tensor(z2[:, 0:HB], z[:, 0:HB], z[:, 0:HB],
                                    Alu.mult)
            nc.vector.tensor_tensor(z3[:, 0:HB], z2[:, 0:HB], z[:, 0:HB],
                                    Alu.mult)

            nc.gpsimd.tensor_tensor(xc2[:], x_sb[:], x_sb[:], Alu.mult)
            nc.gpsimd.tensor_scalar(z[:, ts(5, N_LOC)], x_sb[:],
                                    mprime[5], 0.0, Alu.subtract, Alu.max)
            nc.gpsimd.tensor_tensor(xc3[:], xc2[:], x_sb[:], Alu.mult)
            nc.gpsimd.tensor_tensor(z2[:, HB:], z[:, HB:], z[:, HB:],
                                    Alu.mult)
            nc.gpsimd.tensor_tensor(z3[:, HB:], z2[:, HB:], z[:, HB:],
                                    Alu.mult)

            # f16 bias column (rides the A transfer) -> f32 for the DVE
            # tensor_scalar per-partition operand; off the critical path.
            bias_f32 = pool.tile([D_OUT, 1], f32, tag="biasf32")
            nc.gpsimd.tensor_copy(bias_f32[:],
                                  a_sb[:, _BIAS_COL:_BIAS_COL + 1])

            # ---- contraction: 15 accumulating fp16 matmuls, emission order
            # [p1,p2,p3, s1..s5, c1..c5, s6, c6] matches atom readiness and
            # the A-part split (first 9 from part 1).
            H = ([x_sb[:], xc2[:], xc3[:]]
                 + [z2[:, ts(i, N_LOC)] for i in range(5)]
                 + [z3[:, ts(i, N_LOC)] for i in range(5)]
                 + [z2[:, ts(5, N_LOC)], z3[:, ts(5, N_LOC)]])
            psum = ppool.tile([D_OUT, N_LOC], f32, tag="acc")
            for c in range(N_CHUNKS):
                c0, c1 = _chunk_cols(c)
                nc.tensor.matmul(psum[:], lhsT=a_sb[:, c0:c1], rhs=H[c],
                                 start=(c == 0), stop=(c == N_CHUNKS - 1))

            # PSUM -> SBUF on the DVE, adding the per-o bias column (rides
            # the A transfer; per-partition scalar operand).
            out_sb = pool.tile([D_OUT, N_LOC], f16, tag="out")
            nc.vector.tensor_scalar(out_sb[:], psum[:], bias_f32[:], None,
                                    Alu.add)
            nc.sync.dma_start(out_t[:], out_sb[:])

    nc.compile()
    return nc


def _get_module():
    if "nc" not in _STATE:
        _STATE["nc"] = _build_module()
    return _STATE["nc"]


def _run(x, coeffs, weights, bias, trace=False, tmpdir=None):
    from concourse import bass_utils

    nc = _get_module()
    acat = _prepack(coeffs, weights, bias)
    xT = np.ascontiguousarray(
        np.clip(x, -1.0, 1.0).astype(np.float16).T)            # [j, N]
    in_maps = [
        {"xt": np.ascontiguousarray(xT[:, i * N_LOC:(i + 1) * N_LOC]),
         "acat": acat}
        for i in range(N_CORES)
    ]
    res = bass_utils.run_bass_kernel_spmd(
        nc, in_maps, core_ids=list(range(N_CORES)), trace=trace,
        tmpdir=tmpdir)
    out = np.concatenate([res.results[i]["out_t"] for i in range(N_CORES)],
                         axis=1).T.astype(np.float32)           # [N, o]
    return np.ascontiguousarray(out), res


def kernel(x, coeffs, weights, bias):
    out, _ = _run(np.asarray(x), np.asarray(coeffs), np.asarray(weights),
                  np.asarray(bias))
    return out


# revision 7
# speedup vs baseline: 1.1192x; 1.1192x over previous
"""KAN layer (Catmull-Rom spline edges) as a single-matmul Trainium2 kernel.

Math:
  out[n,o] = sum_j w[o,j] * s_oj(x[n,j]) + bias[o],  s_oj = Catmull-Rom spline
  with K=8 uniform knots on [-1,1].  Each edge spline is decomposed into
  15 atom-chunks (near-side truncated-power basis; 5-tap stencils annihilate
  cubics so the decomposition is well-conditioned):
      out = sum_c  Acol_c^T @ H_c   + bias (added in the PSUM->SBUF copy)
  H atoms: xc, xc^2, xc^3, z_s^2, z_s^3 (s=1..6)
      z_s = min(xc - m'_s, 0) for s<=3, max(xc - m'_s, 0) for s>=4,
            m'_s = (s-3.5)/3.5
  Boundary atoms hD=(xc+1)z1^2, hE=z6^2(3.5 z6-1) are folded into the z1/z6
  square+cube chunks (exact identities on the truncated supports).
  A-side (pure weight prepack) on host in fp16; x is clamped to [-1,1] and
  cast fp16 on host.  Data-parallel over N across 8 NeuronCores.

Perf structure (v3):
  - ALL input DMAs ride the ACT HWDGE ring in need-order (x, A1, A2): the
    16 SDMA engines drain one transfer's batch before switching rings, so
    multi-ring "parallelism" just reorders transfers; a single ring gives
    strict FIFO.  Output rides the idle SP ring.
  - A split 10+5 chunks so the PE can start ~1us before the tail chunks
    land; emission order matches atom readiness.
  - Every SBUF tile has a single writing engine (cross-engine concurrent
    writes to one tile measured 5-10x slowdowns on DVE/GPSIMD ops).
  - ACT does only the z456 square block; its 1.3us act-table load is
    prefetched via a dummy activation right after the DMA issues.
  - bias rides the A transfer and is added during the DVE PSUM->SBUF copy;
    fp16 output DMA (host upcasts).
"""
import numpy as np
from math import comb

N, D_IN, D_OUT, K = 1024, 128, 128, 8
N_CORES = 8
N_LOC = N // N_CORES
N_CHUNKS = 15

_A_COEF = {-2: 0.5, -1: -2.0, 0: 3.0, 1: -2.0, 2: 0.5}
_B_COEF = {-2: -0.5, -1: 1.0, 0: 0.0, 1: -1.0, 2: 0.5}

# build order: [p1,p2,p3, s1..s6, c1..c6]
# emission order (matmul stream / acat column order):
#   [p1,p2,p3, s1,s2,s3, c1,c2,c3, s4, | s5,s6, c4,c5,c6]
_EMIT = [0, 1, 2, 3, 4, 5, 9, 10, 11, 6, 7, 8, 12, 13, 14]
_NC_A1 = 10                     # chunks 0..9 of emission order ride part 1
_BIAS_COL = _NC_A1 * D_OUT      # bias column index inside acat
_PAD = 8                        # bias col + 7 pad cols, keeps 16B alignment
_A1_W = _NC_A1 * D_OUT + _PAD   # part-1 width (1288)
_ACAT_W = N_CHUNKS * D_OUT + _PAD  # 1928

_STATE = {}


def _chunk_cols(c):
    """acat column range of emission-chunk c."""
    if c < _NC_A1:
        s = c * D_OUT
    else:
        s = _A1_W + (c - _NC_A1) * D_OUT
    return s, s + D_OUT


def _poly_xc(s, p):
    """coeffs of (t-s)^p in powers of xc (const..xc^3), t = 3.5*xc + 3.5."""
    c = np.zeros(4)
    for i in range(p + 1):
        c[i] = comb(p, i) * (3.5 ** i) * ((3.5 - s) ** (p - i))
    return c


def _prepack(coeffs, weights, bias):
    """Host weight prepack -> acat fp16 [j, 1928] (emission order + bias)."""
    Ap = (coeffs.astype(np.float64) * weights.astype(np.float64)[:, :, None]
          ).transpose(1, 2, 0)                                   # [j,k,o]
    poly = np.zeros((4, D_IN, D_OUT))
    cube = np.zeros((6, D_IN, D_OUT))
    sq = np.zeros((6, D_IN, D_OUT))
    for k in range(K):
        for r in (-2, -1, 0, 1, 2):
            s = k + r
            ar, br = _A_COEF[r], _B_COEF[r]
            if s >= 7:
                continue
            if s <= 3:
                # a(t-s)_+^3 + b(t-s)_+^2
                #   = [a(t-s)^3 + b(t-s)^2] + a*(s-t)_+^3 - b*(s-t)_+^2
                poly += (ar * _poly_xc(s, 3) + br * _poly_xc(s, 2)
                         )[:, None, None] * Ap[:, k, :][None]
                if s >= 1:
                    # z_s = min(.,0): (s-t)_+^3 = -42.875 z^3,
                    #                 (s-t)_+^2 =  12.25 z^2
                    cube[s - 1] += -42.875 * ar * Ap[:, k, :]
                    sq[s - 1] += -12.25 * br * Ap[:, k, :]
            else:
                # z_s = max(.,0): (t-s)_+^3 = 42.875 z^3, (t-s)_+^2 = 12.25 z^2
                cube[s - 1] += 42.875 * ar * Ap[:, k, :]
                sq[s - 1] += 12.25 * br * Ap[:, k, :]
    D_col = -21.4375 * Ap[:, 0, :]       # atom (xc+1)*z_1^2
    E_col = 6.125 * Ap[:, 7, :]          # atom z_6^2*(3.5 z_6 - 1)
    # Fold the boundary atoms into existing chunks (exact identities on the
    # truncated supports):  hD = (xc+1) z1^2 = z1^3 + (1+m'_1) z1^2
    #                       hE = z6^2 (3.5 z6 - 1) = 3.5 z6^3 - z6^2
    m1 = (1 - 3.5) / 3.5
    cube[0] += D_col
    sq[0] += (1.0 + m1) * D_col
    cube[5] += 3.5 * E_col
    sq[5] -= E_col
    A = np.stack([poly[1], poly[2], poly[3], sq[0], sq[1], sq[2],
                  sq[3], sq[4], sq[5],
                  cube[0], cube[1], cube[2], cube[3], cube[4], cube[5]]
                 )                                               # [15,j,o]
    acat = np.zeros((D_IN, _ACAT_W), dtype=np.float16)
    for e, b in enumerate(_EMIT):
        c0, c1 = _chunk_cols(e)
        acat[:, c0:c1] = A[b].astype(np.float16)
    bias_full = (bias.astype(np.float64) + poly[0].sum(axis=0)
                 ).astype(np.float16)                            # [o]
    acat[:, _BIAS_COL] = bias_full                  # partition p holds bias[p]
    return np.ascontiguousarray(acat)


def _build_module():
    import concourse.bacc as bacc
    import concourse.bass as bass
    import concourse.mybir as mybir
    from concourse import tile

    f32 = mybir.dt.float32
    f16 = mybir.dt.float16
    Alu = mybir.AluOpType
    Act = mybir.ActivationFunctionType
    ts = bass.ts

    # Skip the all-engine barrier Bass.__init__ emits after the const-AP
    # memsets (~0.5us before the first DMA issue can happen).  The only
    # const-AP reader here is the ACT square (bias=0.0 const), gated ~3us
    # later by the x DMA + z computation, so the memsets always win.
    _orig_barrier = bass.Bass.all_engine_barrier

    def _skip_once(self, *a, **k):
        bass.Bass.all_engine_barrier = _orig_barrier
        return None

    bass.Bass.all_engine_barrier = _skip_once
    try:
        nc = bacc.Bacc("TRN2", target_bir_lowering=False, debug=False,
                       enable_asserts=False, num_devices=N_CORES)
    finally:
        bass.Bass.all_engine_barrier = _orig_barrier
    xt = nc.dram_tensor("xt", [D_IN, N_LOC], f16, kind="ExternalInput").ap()
    acat = nc.dram_tensor("acat", [D_IN, _ACAT_W], f16,
                          kind="ExternalInput").ap()
    out_t = nc.dram_tensor("out_t", [D_OUT, N_LOC], f16,
                           kind="ExternalOutput").ap()

    mprime = [(s - 3.5) / 3.5 for s in range(1, 7)]
    HB = 3 * N_LOC  # half-block of z columns

    with tile.TileContext(nc) as tc:
        with (
            tc.tile_pool(name="sbuf", bufs=1) as pool,
            tc.tile_pool(name="psum", bufs=1, space="PSUM") as ppool,
        ):
            x_sb = pool.tile([D_IN, N_LOC], f16, tag="x")
            a_sb = pool.tile([D_IN, _ACAT_W], f16, tag="acat")

            # ---- Input DMAs, ALL on the ACT HWDGE ring, in need-order.
            nc.scalar.dma_start(x_sb[:], xt[:])
            nc.scalar.dma_start(a_sb[:, 0:_A1_W], acat[:, 0:_A1_W])
            nc.scalar.dma_start(a_sb[:, _A1_W:_ACAT_W], acat[:, _A1_W:_ACAT_W])

            # Dummy activation right after the DMA issues: pulls the ACT
            # table load (~1.3us) off the critical path (it would otherwise
            # run right before the first real ACT op).
            dm = pool.tile([1, 1], f16, tag="dm")
            nc.gpsimd.memset(dm[:], 0.0)
            dmo = pool.tile([1, 1], f16, tag="dmo")
            nc.scalar.activation(dmo[:], dm[:], Act.Square)

            # ---- x-side atoms (all fp16; x arrives pre-clamped) ----
            # Single writer per tile: z/z2a/z3/xc3 DVE, z2b ACT, xc2 GPSIMD.
            z = pool.tile([D_IN, 6 * N_LOC], f16, tag="z")
            z2a = pool.tile([D_IN, HB], f16, tag="z2a")
            z2b = pool.tile([D_IN, HB], f16, tag="z2b")
            z3 = pool.tile([D_IN, 6 * N_LOC], f16, tag="z3")
            xc2 = pool.tile([D_IN, N_LOC], f16, tag="xc2")
            xc3 = pool.tile([D_IN, N_LOC], f16, tag="xc3")

            nc.gpsimd.tensor_tensor(xc2[:], x_sb[:], x_sb[:], Alu.mult)
            for i in range(3):
                nc.vector.tensor_scalar(z[:, ts(i, N_LOC)], x_sb[:],
                                        mprime[i], 0.0, Alu.subtract, Alu.min)
            for i in (3, 4, 5):
                nc.vector.tensor_scalar(z[:, ts(i, N_LOC)], x_sb[:],
                                        mprime[i], 0.0, Alu.subtract, Alu.max)
            nc.scalar.activation(z2b[:], z[:, HB:], Act.Square)
            nc.vector.tensor_tensor(z2a[:], z[:, 0:HB], z[:, 0:HB], Alu.mult)
            nc.vector.tensor_tensor(xc3[:], xc2[:], x_sb[:], Alu.mult)
            nc.vector.tensor_tensor(z3[:, 0:HB], z2a[:], z[:, 0:HB], Alu.mult)
            nc.vector.tensor_tensor(z3[:, HB:], z2b[:], z[:, HB:], Alu.mult)

            # f16 bias column (rides the A transfer) -> f32 for the DVE
            # tensor_scalar per-partition operand; off the critical path.
            bias_f32 = pool.tile([D_OUT, 1], f32, tag="biasf32")
            nc.gpsimd.tensor_copy(bias_f32[:],
                                  a_sb[:, _BIAS_COL:_BIAS_COL + 1])

            # ---- contraction: 15 accumulating fp16 matmuls, emission order
            # [p1,p2,p3, s1,s2,s3, c1,c2,c3, s4, s5,s6, c4,c5,c6] matches
            # atom readiness and the A-part split (first 10 from part 1).
            # emission H sources: p1,p2,p3, s1,s2,s3 (z2a), c1,c2,c3 (z3 lo),
            # s4 (z2b0), s5,s6 (z2b1,2), c4,c5,c6 (z3 hi)
            emis_H = [x_sb[:], xc2[:], xc3[:],
                      z2a[:, ts(0, N_LOC)], z2a[:, ts(1, N_LOC)],
                      z2a[:, ts(2, N_LOC)],
                      z3[:, ts(0, N_LOC)], z3[:, ts(1, N_LOC)],
                      z3[:, ts(2, N_LOC)],
                      z2b[:, ts(0, N_LOC)],
                      z2b[:, ts(1, N_LOC)], z2b[:, ts(2, N_LOC)],
                      z3[:, ts(3, N_LOC)], z3[:, ts(4, N_LOC)],
                      z3[:, ts(5, N_LOC)]]
            psum = ppool.tile([D_OUT, N_LOC], f32, tag="acc")
            for c in range(N_CHUNKS):
                c0, c1 = _chunk_cols(c)
                nc.tensor.matmul(psum[:], lhsT=a_sb[:, c0:c1], rhs=emis_H[c],
                                 start=(c == 0), stop=(c == N_CHUNKS - 1))

            # PSUM -> SBUF on the DVE, adding the per-o bias column.
            out_sb = pool.tile([D_OUT, N_LOC], f16, tag="out")
            nc.vector.tensor_scalar(out_sb[:], psum[:], bias_f32[:], None,
                                    Alu.add)
            nc.sync.dma_start(out_t[:], out_sb[:])

    nc.compile()
    return nc


def _get_module():
    if "nc" not in _STATE:
        _STATE["nc"] = _build_module()
    return _STATE["nc"]


def _run(x, coeffs, weights, bias, trace=False, tmpdir=None):
    from concourse import bass_utils

    nc = _get_module()
    acat = _prepack(coeffs, weights, bias)
    xT = np.ascontiguousarray(
        np.clip(x, -1.0, 1.0).astype(np.float16).T)            # [j, N]
    in_maps = [
        {"xt": np.ascontiguousarray(xT[:, i * N_LOC:(i + 1) * N_LOC]),
         "acat": acat}
        for i in range(N_CORES)
    ]
    res = bass_utils.run_bass_kernel_spmd(
        nc, in_maps, core_ids=list(range(N_CORES)), trace=trace,
        tmpdir=tmpdir)
    out = np.concatenate([res.results[i]["out_t"] for i in range(N_CORES)],
                         axis=1).T.astype(np.float32)           # [N, o]
    return np.ascontiguousarray(out), res


def kernel(x, coeffs, weights, bias):
    out, _ = _run(np.asarray(x), np.asarray(coeffs), np.asarray(weights),
                  np.asarray(bias))
    return out


# revision 10
# speedup vs baseline: 1.1511x; 1.0286x over previous
"""KAN layer (Catmull-Rom spline edges) as a single-matmul Trainium2 kernel.

Math:
  out[n,o] = sum_j w[o,j] * s_oj(x[n,j]) + bias[o],  s_oj = Catmull-Rom spline
  with K=8 uniform knots on [-1,1].  Each edge spline is decomposed into
  15 atom-chunks (near-side truncated-power basis; 5-tap stencils annihilate
  cubics so the decomposition is well-conditioned):
      out = sum_c  Acol_c^T @ H_c   + bias (added in the PSUM->SBUF copy)
  H atoms: xc, xc^2, xc^3, z_s^2, z_s^3 (s=1..6)
      z_s = min(xc - m'_s, 0) for s<=3, max(xc - m'_s, 0) for s>=4,
            m'_s = (s-3.5)/3.5
  Boundary atoms hD=(xc+1)z1^2, hE=z6^2(3.5 z6-1) are folded into the z1/z6
  square+cube chunks (exact identities on the truncated supports).
  A-side (pure weight prepack) on host in fp16; x is clamped to [-1,1] and
  cast fp16 on host.  Data-parallel over N across 8 NeuronCores.

Perf structure (v3):
  - ALL input DMAs ride the ACT HWDGE ring in need-order (x, A1, A2): the
    16 SDMA engines drain one transfer's batch before switching rings, so
    multi-ring "parallelism" just reorders transfers; a single ring gives
    strict FIFO.  Output rides the idle SP ring.
  - A split 10+5 chunks so the PE can start ~1us before the tail chunks
    land; emission order matches atom readiness.
  - Every SBUF tile has a single writing engine (cross-engine concurrent
    writes to one tile measured 5-10x slowdowns on DVE/GPSIMD ops).
  - ACT does only the z456 square block; its 1.3us act-table load is
    prefetched via a dummy activation right after the DMA issues.
  - bias rides the A transfer and is added during the DVE PSUM->SBUF copy;
    fp16 output DMA (host upcasts).
"""
import numpy as np
from math import comb

N, D_IN, D_OUT, K = 1024, 128, 128, 8
N_CORES = 8
N_LOC = N // N_CORES
N_CHUNKS = 15

_A_COEF = {-2: 0.5, -1: -2.0, 0: 3.0, 1: -2.0, 2: 0.5}
_B_COEF = {-2: -0.5, -1: 1.0, 0: 0.0, 1: -1.0, 2: 0.5}

# build order: [p1,p2,p3, s1..s6, c1..c6]
# emission order (matmul stream / acat column order):
#   [p1,p2,p3, s1,s2,s3, | c1,c2,c3, s4,s5, | s6, c4,c5,c6]
_EMIT = [0, 1, 2, 3, 4, 5, 9, 10, 11, 6, 7, 8, 12, 13, 14]
_S1, _S2 = 6, 11                # A-part split points (emission indices)
_BIAS_COL = _S2 * D_OUT         # bias column index inside acat (end of part 2)
_PAD = 8                        # bias col + 7 pad cols, keeps 16B alignment
_P2_END = _S2 * D_OUT + _PAD    # part-2 end (1416)
_ACAT_W = N_CHUNKS * D_OUT + _PAD  # 1928

_STATE = {}


def _chunk_cols(c):
    """acat column range of emission-chunk c."""
    if c < _S2:
        s = c * D_OUT
    else:
        s = _P2_END + (c - _S2) * D_OUT
    return s, s + D_OUT


def _poly_xc(s, p):
    """coeffs of (t-s)^p in powers of xc (const..xc^3), t = 3.5*xc + 3.5."""
    c = np.zeros(4)
    for i in range(p + 1):
        c[i] = comb(p, i) * (3.5 ** i) * ((3.5 - s) ** (p - i))
    return c


def _prepack(coeffs, weights, bias):
    """Host weight prepack -> acat fp16 [j, 1928] (emission order + bias)."""
    Ap = (coeffs.astype(np.float64) * weights.astype(np.float64)[:, :, None]
          ).transpose(1, 2, 0)                                   # [j,k,o]
    poly = np.zeros((4, D_IN, D_OUT))
    cube = np.zeros((6, D_IN, D_OUT))
    sq = np.zeros((6, D_IN, D_OUT))
    for k in range(K):
        for r in (-2, -1, 0, 1, 2):
            s = k + r
            ar, br = _A_COEF[r], _B_COEF[r]
            if s >= 7:
                continue
            if s <= 3:
                # a(t-s)_+^3 + b(t-s)_+^2
                #   = [a(t-s)^3 + b(t-s)^2] + a*(s-t)_+^3 - b*(s-t)_+^2
                poly += (ar * _poly_xc(s, 3) + br * _poly_xc(s, 2)
                         )[:, None, None] * Ap[:, k, :][None]
                if s >= 1:
                    # z_s = min(.,0): (s-t)_+^3 = -42.875 z^3,
                    #                 (s-t)_+^2 =  12.25 z^2
                    cube[s - 1] += -42.875 * ar * Ap[:, k, :]
                    sq[s - 1] += -12.25 * br * Ap[:, k, :]
            else:
                # z_s = max(.,0): (t-s)_+^3 = 42.875 z^3, (t-s)_+^2 = 12.25 z^2
                cube[s - 1] += 42.875 * ar * Ap[:, k, :]
                sq[s - 1] += 12.25 * br * Ap[:, k, :]
    D_col = -21.4375 * Ap[:, 0, :]       # atom (xc+1)*z_1^2
    E_col = 6.125 * Ap[:, 7, :]          # atom z_6^2*(3.5 z_6 - 1)
    # Fold the boundary atoms into existing chunks (exact identities on the
    # truncated supports):  hD = (xc+1) z1^2 = z1^3 + (1+m'_1) z1^2
    #                       hE = z6^2 (3.5 z6 - 1) = 3.5 z6^3 - z6^2
    m1 = (1 - 3.5) / 3.5
    cube[0] += D_col
    sq[0] += (1.0 + m1) * D_col
    cube[5] += 3.5 * E_col
    sq[5] -= E_col
    A = np.stack([poly[1], poly[2], poly[3], sq[0], sq[1], sq[2],
                  sq[3], sq[4], sq[5],
                  cube[0], cube[1], cube[2], cube[3], cube[4], cube[5]]
                 )                                               # [15,j,o]
    acat = np.zeros((D_IN, _ACAT_W), dtype=np.float16)
    for e, b in enumerate(_EMIT):
        c0, c1 = _chunk_cols(e)
        acat[:, c0:c1] = A[b].astype(np.float16)
    bias_full = (bias.astype(np.float64) + poly[0].sum(axis=0)
                 ).astype(np.float16)                            # [o]
    acat[:, _BIAS_COL] = bias_full                  # partition p holds bias[p]
    return np.ascontiguousarray(acat)


def _build_module():
    import concourse.bacc as bacc
    import concourse.bass as bass
    import concourse.mybir as mybir
    from concourse import tile

    f32 = mybir.dt.float32
    f16 = mybir.dt.float16
    Alu = mybir.AluOpType
    Act = mybir.ActivationFunctionType
    ts = bass.ts

    # Skip the all-engine barrier Bass.__init__ emits after the const-AP
    # memsets (~0.5us before the first DMA issue can happen).  The only
    # const-AP reader here is the ACT square (bias=0.0 const), gated ~3us
    # later by the x DMA + z computation, so the memsets always win.
    _orig_barrier = bass.Bass.all_engine_barrier

    def _skip_once(self, *a, **k):
        bass.Bass.all_engine_barrier = _orig_barrier
        return None

    bass.Bass.all_engine_barrier = _skip_once
    try:
        nc = bacc.Bacc("TRN2", target_bir_lowering=False, debug=False,
                       enable_asserts=False, num_devices=N_CORES)
    finally:
        bass.Bass.all_engine_barrier = _orig_barrier
    xt = nc.dram_tensor("xt", [D_IN, N_LOC], f16, kind="ExternalInput").ap()
    acat = nc.dram_tensor("acat", [D_IN, _ACAT_W], f16,
                          kind="ExternalInput").ap()
    out_t = nc.dram_tensor("out_t", [D_OUT, N_LOC], f16,
                           kind="ExternalOutput").ap()

    mprime = [(s - 3.5) / 3.5 for s in range(1, 7)]
    HB = 3 * N_LOC  # half-block of z columns

    with tile.TileContext(nc) as tc:
        with (
            tc.tile_pool(name="sbuf", bufs=1) as pool,
            tc.tile_pool(name="psum", bufs=1, space="PSUM") as ppool,
        ):
            x_sb = pool.tile([D_IN, N_LOC], f16, tag="x")
            a_sb = pool.tile([D_IN, _ACAT_W], f16, tag="acat")

            # ---- Input DMAs, ALL on the ACT HWDGE ring, in need-order.
            # A is split 6/5/4 chunks so the PE starts ~1us before the tail
            # chunks land (pipelined with the matmul stream).
            _B1 = _S1 * D_OUT
            nc.scalar.dma_start(x_sb[:], xt[:])
            nc.scalar.dma_start(a_sb[:, 0:_B1], acat[:, 0:_B1])
            nc.scalar.dma_start(a_sb[:, _B1:_P2_END], acat[:, _B1:_P2_END])
            nc.scalar.dma_start(a_sb[:, _P2_END:_ACAT_W],
                                acat[:, _P2_END:_ACAT_W])

            # Dummy activation right after the DMA issues: pulls the ACT
            # table load (~1.3us) off the critical path (it would otherwise
            # run right before the first real ACT op).
            dm = pool.tile([1, 1], f16, tag="dm")
            nc.gpsimd.memset(dm[:], 0.0)
            dmo = pool.tile([1, 1], f16, tag="dmo")
            nc.scalar.activation(dmo[:], dm[:], Act.Square)

            # ---- x-side atoms (all fp16; x arrives pre-clamped) ----
            # Single writer per tile: z/z2a/z3/xc3 DVE, z2b ACT, xc2 GPSIMD.
            z = pool.tile([D_IN, 6 * N_LOC], f16, tag="z")
            z2a = pool.tile([D_IN, HB], f16, tag="z2a")
            z2b = pool.tile([D_IN, HB], f16, tag="z2b")
            z3 = pool.tile([D_IN, 6 * N_LOC], f16, tag="z3")
            xc2 = pool.tile([D_IN, N_LOC], f16, tag="xc2")
            xc3 = pool.tile([D_IN, N_LOC], f16, tag="xc3")

            nc.gpsimd.tensor_tensor(xc2[:], x_sb[:], x_sb[:], Alu.mult)
            nc.gpsimd.tensor_tensor(xc3[:], xc2[:], x_sb[:], Alu.mult)
            for i in range(3):
                nc.vector.tensor_scalar(z[:, ts(i, N_LOC)], x_sb[:],
                                        mprime[i], 0.0, Alu.subtract, Alu.min)
            for i in (3, 4, 5):
                nc.vector.tensor_scalar(z[:, ts(i, N_LOC)], x_sb[:],
                                        mprime[i], 0.0, Alu.subtract, Alu.max)
            nc.scalar.activation(z2b[:], z[:, HB:], Act.Square)
            nc.vector.tensor_tensor(z2a[:], z[:, 0:HB], z[:, 0:HB], Alu.mult)
            nc.vector.tensor_tensor(z3[:, 0:HB], z2a[:], z[:, 0:HB], Alu.mult)
            nc.vector.tensor_tensor(z3[:, HB:], z2b[:], z[:, HB:], Alu.mult)

            # f16 bias column (rides the A transfer) -> f32 for the DVE
            # tensor_scalar per-partition operand; off the critical path.
            bias_f32 = pool.tile([D_OUT, 1], f32, tag="biasf32")
            nc.gpsimd.tensor_copy(bias_f32[:],
                                  a_sb[:, _BIAS_COL:_BIAS_COL + 1])

            # ---- contraction: 15 accumulating fp16 matmuls, emission order
            # [p1,p2,p3, s1,s2,s3, c1,c2,c3, s4, s5,s6, c4,c5,c6] matches
            # atom readiness and the A-part split (first 10 from part 1).
            # emission H sources: p1,p2,p3, s1,s2,s3 (z2a), c1,c2,c3 (z3 lo),
            # s4 (z2b0), s5,s6 (z2b1,2), c4,c5,c6 (z3 hi)
            emis_H = [x_sb[:], xc2[:], xc3[:],
                      z2a[:, ts(0, N_LOC)], z2a[:, ts(1, N_LOC)],
                      z2a[:, ts(2, N_LOC)],
                      z3[:, ts(0, N_LOC)], z3[:, ts(1, N_LOC)],
                      z3[:, ts(2, N_LOC)],
                      z2b[:, ts(0, N_LOC)],
                      z2b[:, ts(1, N_LOC)], z2b[:, ts(2, N_LOC)],
                      z3[:, ts(3, N_LOC)], z3[:, ts(4, N_LOC)],
                      z3[:, ts(5, N_LOC)]]
            psum = ppool.tile([D_OUT, N_LOC], f32, tag="acc")
            for c in range(N_CHUNKS):
                c0, c1 = _chunk_cols(c)
                nc.tensor.matmul(psum[:], lhsT=a_sb[:, c0:c1], rhs=emis_H[c],
                                 start=(c == 0), stop=(c == N_CHUNKS - 1))

            # PSUM -> SBUF on the DVE, adding the per-o bias column.
            out_sb = pool.tile([D_OUT, N_LOC], f16, tag="out")
            nc.vector.tensor_scalar(out_sb[:], psum[:], bias_f32[:], None,
                                    Alu.add)
            nc.sync.dma_start(out_t[:], out_sb[:])

    nc.compile()
    return nc


def _get_module():
    if "nc" not in _STATE:
        _STATE["nc"] = _build_module()
    return _STATE["nc"]


def _run(x, coeffs, weights, bias, trace=False, tmpdir=None):
    from concourse import bass_utils

    nc = _get_module()
    acat = _prepack(coeffs, weights, bias)
    xT = np.ascontiguousarray(
        np.clip(x, -1.0, 1.0).astype(np.float16).T)            # [j, N]
    in_maps = [
        {"xt": np.ascontiguousarray(xT[:, i * N_LOC:(i + 1) * N_LOC]),
         "acat": acat}
        for i in range(N_CORES)
    ]
    res = bass_utils.run_bass_kernel_spmd(
        nc, in_maps, core_ids=list(range(N_CORES)), trace=trace,
        tmpdir=tmpdir)
    out = np.concatenate([res.results[i]["out_t"] for i in range(N_CORES)],
                         axis=1).T.astype(np.float32)           # [N, o]
    return np.ascontiguousarray(out), res


def kernel(x, coeffs, weights, bias):
    out, _ = _run(np.asarray(x), np.asarray(coeffs), np.asarray(weights),
                  np.asarray(bias))
    return out
